# revision 7
# baseline (speedup 1.0000x reference)
"""CenterPNLoss on 8 TRN2 NeuronCores.

Math: the reference builds two 8192x8192 distance matrices between
per-row class centers and all points, then does masked row reductions.
Both matrices have only <=1024 unique rows (one per identity g), and the
masked sums only ever need, for each (center g, label h), the sum of
distances from center g to all points with label h:

    R2[g, h] = sum_{j: targets[j]==h} sqrt(||c_g||^2 + ||x_j||^2 - 2 c_g.x_j)

From R2 (shape [1024, 1024], per modality) every reference quantity is a
cheap gather/sum over 8192 rows, done on the host in f64.

Device work per core (label-sharded: core c owns labels [128c, 128c+128)):
  columns = points sorted by label, zero-padded to Pw per label group.
  psum[g, j] = n_x[j] (K=1 matmul vs ones) - 2 c_g.x_j (two K=128 matmuls)
  d = ACT Sqrt(psum + bias=||c_g||^2)   (per-partition bias)
  R2 chunk = DVE grouped reduce of d over each Pw-wide label group.
Pad columns (x=0, n_x=0) contribute sqrt(||c_g||^2) exactly; the host
subtracts npad[h]*sqrt(nr[g]) afterwards.

No clip-before-sqrt on device: d >= ~100 for randn data with mean-of-4
centers, and pad columns give exactly nr[g] >= 0, so NaN is impossible.
"""

import os
import sys
from contextlib import ExitStack

import numpy as np

sys.path.insert(0, "/opt/trn_rl_repo")

import concourse.bass as bass
import concourse.tile as tile
from concourse import bacc, mybir
from concourse.bass_utils import run_bass_kernel_spmd

N = 8192
D = 256
HALF = N // 2
NSEG = 1024
NCORES = 8
GPC = NSEG // NCORES  # label groups per core: 128

# Matmul operand dtype. float32 = safe baseline (4 cycles/row on PE).
# float32r = full-rate fp32 matmul mode (1 cycle/row when N>=256).
MM_DT = mybir.dt.float32

_nc_cache: dict = {}
last_result = None  # BassKernelResults of the most recent run (for test.py)


def build_nc(G: int, Pw: int, mm_dt=MM_DT):
    """One-core SPMD program: [257, G] rhs shard -> [1024, 256] R2 shard."""
    assert G % 512 == 0 and 512 % Pw == 0
    ntile = G // 512       # 512-column tiles
    gtile = 512 // Pw      # label groups per column tile
    f32 = mybir.dt.float32

    # Bacc (not plain Bass): its finalize() runs move_matmul_waits_to_
    # ldweights + generate_event_semaphores, without which walrus rejects
    # Tile-scheduled matmuls ("Too many sync wait commands").
    nc = bacc.Bacc()
    rhs_d = nc.declare_dram_parameter("rhs", [257, G], mm_dt, isOutput=False)
    lhsR_d = nc.declare_dram_parameter("lhsR", [D, NSEG], mm_dt, isOutput=False)
    lhsI_d = nc.declare_dram_parameter("lhsI", [D, NSEG], mm_dt, isOutput=False)
    nr_d = nc.declare_dram_parameter("nr", [128, 16], f32, isOutput=False)
    r2_d = nc.declare_dram_parameter("r2", [NSEG, 2 * GPC], f32, isOutput=True)

    with tile.TileContext(nc) as tc, ExitStack() as ctx:
        const = ctx.enter_context(tc.tile_pool(name="const", bufs=1))
        psum = ctx.enter_context(tc.tile_pool(name="psum", bufs=4, space="PSUM"))
        dpool = ctx.enter_context(tc.tile_pool(name="d", bufs=4))
        opool = ctx.enter_context(tc.tile_pool(name="o", bufs=2))

        rhs0 = const.tile([128, G], mm_dt, tag="rhs0")
        rhs1 = const.tile([128, G], mm_dt, tag="rhs1")
        nx = const.tile([1, G], mm_dt, tag="nx")
        lhs = {}
        for mod, src in ((0, lhsR_d), (1, lhsI_d)):
            for kb in range(2):
                t = const.tile([128, NSEG], mm_dt, tag=f"lhs{mod}{kb}")
                nc.sync.dma_start(out=t[:], in_=src[kb * 128 : (kb + 1) * 128, :])
                lhs[mod, kb] = t
        nr_t = const.tile([128, 16], f32, tag="nr")
        ones_t = const.tile([1, 128], mm_dt, tag="ones")

        nc.sync.dma_start(out=rhs0[:], in_=rhs_d[0:128, :])
        nc.sync.dma_start(out=rhs1[:], in_=rhs_d[128:256, :])
        nc.sync.dma_start(out=nx[:1, :], in_=rhs_d[256:257, :])
        nc.sync.dma_start(out=nr_t[:], in_=nr_d[:, :])
        nc.vector.memset(ones_t[:1, :], 1.0)

        for m in range(8):
            out_t = opool.tile([128, 2 * GPC], f32)
            for mod in range(2):
                bias = nr_t[:, mod * 8 + m : mod * 8 + m + 1]
                for t in range(ntile):
                    cs = bass.ts(t, 512)
                    ps = psum.tile([128, 512], f32)
                    nc.tensor.matmul(
                        ps[:], ones_t[:1, :], nx[:1, cs], start=True, stop=False
                    )
                    nc.tensor.matmul(
                        ps[:], lhs[mod, 0][:, bass.ts(m, 128)], rhs0[:, cs],
                        start=False, stop=False,
                    )
                    nc.tensor.matmul(
                        ps[:], lhs[mod, 1][:, bass.ts(m, 128)], rhs1[:, cs],
                        start=False, stop=True,
                    )
                    d_t = dpool.tile([128, 512], f32)
                    # d = max(psum + nr[g], 1e-12); singleton-label centers
                    # equal their point, so psum+nr can round below zero.
                    nc.vector.tensor_scalar(
                        d_t[:], ps[:], bias, 1e-12,
                        op0=mybir.AluOpType.add, op1=mybir.AluOpType.max,
                    )
                    nc.scalar.activation(
                        d_t[:], d_t[:], mybir.ActivationFunctionType.Sqrt,
                    )
                    oc = mod * GPC + t * gtile
                    nc.vector.tensor_reduce(
                        out_t[:, oc : oc + gtile],
                        d_t[:].rearrange("p (g w) -> p g w", w=Pw),
                        axis=mybir.AxisListType.X,
                        op=mybir.AluOpType.add,
                    )
            nc.sync.dma_start(out=r2_d[bass.ts(m, 128), :], in_=out_t[:])
    # Bacc defers register allocation to finalize(); serialize-after-
    # finalize or walrus sees reg_id=-1.
    nc.finalize()
    return nc


def _seg_mean(x_half: np.ndarray, t_half: np.ndarray):
    """f64 segment mean matching jax.ops.segment_sum + max(count,1) divide."""
    cnt = np.bincount(t_half, minlength=NSEG)
    sums = np.zeros((NSEG, D), np.float64)
    order = np.argsort(t_half, kind="stable")
    xs = x_half[order].astype(np.float64)
    ts_sorted = t_half[order]
    present = np.nonzero(cnt)[0]
    if len(present):
        starts = np.searchsorted(ts_sorted, present)
        sums[present] = np.add.reduceat(xs, starts, axis=0)
    return (sums / np.maximum(cnt, 1)[:, None]).astype(np.float32), cnt


def prepare(inputs: np.ndarray, targets: np.ndarray):
    """Host data marshaling: centers, sorted/padded rhs, per-core in_maps."""
    x = np.asarray(inputs, np.float32)
    t = np.asarray(targets)
    centerR, _ = _seg_mean(x[:HALF], t[:HALF])
    centerI, _ = _seg_mean(x[HALF:], t[HALF:])
    nrR = np.sum(centerR.astype(np.float64) ** 2, axis=1).astype(np.float32)
    nrI = np.sum(centerI.astype(np.float64) ** 2, axis=1).astype(np.float32)
    n_x = np.sum(x.astype(np.float64) ** 2, axis=1).astype(np.float32)

    cnt_all = np.bincount(t, minlength=NSEG)
    maxc = int(cnt_all.max())
    Pw = 4
    while Pw < maxc:
        Pw *= 2
    assert Pw <= 512, f"label group of {maxc} too large"
    Gt = NSEG * Pw
    G = Gt // NCORES

    starts_pos = np.concatenate([[0], np.cumsum(cnt_all)])[:-1]
    order_all = np.argsort(t, kind="stable")
    ts_all = t[order_all]
    dest = ts_all * Pw + (np.arange(N) - starts_pos[ts_all])
    RHS = np.zeros((257, Gt), np.float32)
    RHS[0:256, dest] = x[order_all].T
    RHS[256, dest] = n_x[order_all]
    npad = (Pw - cnt_all).astype(np.float64)

    nr_dev = np.zeros((128, 16), np.float32)
    for m in range(8):
        nr_dev[:, m] = nrR[m * 128 : (m + 1) * 128]
        nr_dev[:, 8 + m] = nrI[m * 128 : (m + 1) * 128]

    lhsR_dev = np.ascontiguousarray((-2.0 * centerR.T).astype(np.float32))
    lhsI_dev = np.ascontiguousarray((-2.0 * centerI.T).astype(np.float32))
    in_maps = [
        {
            "rhs": np.ascontiguousarray(RHS[:, c * G : (c + 1) * G]),
            "lhsR": lhsR_dev,
            "lhsI": lhsI_dev,
            "nr": nr_dev,
        }
        for c in range(NCORES)
    ]
    host = dict(
        centerR=centerR, centerI=centerI, nrR=nrR, nrI=nrI,
        cnt_all=cnt_all, npad=npad, G=G, Pw=Pw, targets=t,
    )
    return in_maps, host


def finish(core_outs, host) -> np.float32:
    """Assemble R2 shards, pad-correct, and reduce to the scalar loss (f64)."""
    t = host["targets"]
    R2R = np.empty((NSEG, NSEG), np.float64)
    R2I = np.empty((NSEG, NSEG), np.float64)
    for c in range(NCORES):
        R2R[:, c * GPC : (c + 1) * GPC] = core_outs[c][:, :GPC]
        R2I[:, c * GPC : (c + 1) * GPC] = core_outs[c][:, GPC:]
    sqrtR = np.sqrt(host["nrR"].astype(np.float64))
    sqrtI = np.sqrt(host["nrI"].astype(np.float64))
    R2R -= sqrtR[:, None] * host["npad"][None, :]
    R2I -= sqrtI[:, None] * host["npad"][None, :]
    rowsumR = R2R.sum(axis=1)
    rowsumI = R2I.sum(axis=1)

    a = 1.0 / (N - host["cnt_all"][t]).astype(np.float64)
    # cR2[i] = centerR[tR[i mod half]] but cI2[i] = centerI[tI[i mod half]]
    gqR = t[np.arange(N) % HALF]
    gqI = t[HALF + (np.arange(N) % HALF)]
    sumR = float(np.sum(a * (rowsumR[gqR] - R2R[gqR, t])))
    sumI = float(np.sum(a * (rowsumI[gqI] - R2I[gqI, t])))

    diff = host["centerR"][t[:HALF]].astype(np.float64) - host["centerI"][
        t[HALF:]
    ].astype(np.float64)
    s_pc = float(np.sum(np.sqrt(np.sum(diff * diff, axis=1))))
    return np.float32(s_pc / (sumR + sumI - s_pc))


def kernel(inputs: np.ndarray, targets: np.ndarray) -> np.ndarray:
    global last_result
    in_maps, host = prepare(inputs, targets)
    key = (host["G"], host["Pw"], MM_DT)
    if key not in _nc_cache:
        _nc_cache[key] = build_nc(host["G"], host["Pw"], MM_DT)
    nc = _nc_cache[key]
    res = run_bass_kernel_spmd(nc, in_maps, list(range(NCORES)))
    last_result = res
    outs = [res.results[c]["r2"] for c in range(NCORES)]
    return finish(outs, host)


# revision 11
# speedup vs baseline: 1.4187x; 1.4187x over previous
"""CenterPNLoss on 8 TRN2 NeuronCores.

Math: the reference builds two 8192x8192 distance matrices between
per-row class centers and all points, then does masked row reductions.
Both matrices have only <=1024 unique rows (one per identity g), and the
masked sums only ever need, for each (center g, label h), the sum of
distances from center g to all points with label h:

    R2[g, h] = sum_{j: targets[j]==h} sqrt(||c_g||^2 + ||x_j||^2 - 2 c_g.x_j)

From R2 (shape [1024, 1024], per modality) every reference quantity is a
cheap gather/sum over 8192 rows, done on the host in f64.

Device work per core (label-sharded: core c owns labels [128c, 128c+128)):
  columns = points sorted by label, zero-padded to Pw per label group.
  psum[g, j] = n_x[j] (K=1 matmul vs ones) - 2 c_g.x_j (two K=128 matmuls)
  d = ACT Sqrt(psum + bias=||c_g||^2)   (per-partition bias)
  R2 chunk = DVE grouped reduce of d over each Pw-wide label group.
Pad columns (x=0, n_x=0) contribute sqrt(||c_g||^2) exactly; the host
subtracts npad[h]*sqrt(nr[g]) afterwards.

No clip-before-sqrt on device: d >= ~100 for randn data with mean-of-4
centers, and pad columns give exactly nr[g] >= 0, so NaN is impossible.
"""

import os
import sys
from contextlib import ExitStack

import numpy as np

sys.path.insert(0, "/opt/trn_rl_repo")

import concourse.bass as bass
import concourse.tile as tile
from concourse import bacc, mybir
from concourse.bass_utils import run_bass_kernel_spmd

N = 8192
D = 256
HALF = N // 2
NSEG = 1024
NCORES = 8
GPC = NSEG // NCORES  # label groups per core: 128

# Matmul operand dtype. float32 = safe baseline (4 cycles/row on PE).
# float32r = full-rate fp32 matmul mode (1 cycle/row when N>=256).
MM_DT = mybir.dt.float32r

_nc_cache: dict = {}
last_result = None  # BassKernelResults of the most recent run (for test.py)


def build_nc(G: int, Pw: int, mm_dt=MM_DT):
    """One-core SPMD program: [257, G] rhs shard -> [1024, 256] R2 shard."""
    assert G % 512 == 0 and 512 % Pw == 0
    ntile = G // 512       # 512-column tiles
    gtile = 512 // Pw      # label groups per column tile
    f32 = mybir.dt.float32

    # Bacc (not plain Bass): its finalize() runs move_matmul_waits_to_
    # ldweights + generate_event_semaphores, without which walrus rejects
    # Tile-scheduled matmuls ("Too many sync wait commands").
    nc = bacc.Bacc()
    rhs_d = nc.declare_dram_parameter("rhs", [257, G], mm_dt, isOutput=False)
    lhsR_d = nc.declare_dram_parameter("lhsR", [D, NSEG], mm_dt, isOutput=False)
    lhsI_d = nc.declare_dram_parameter("lhsI", [D, NSEG], mm_dt, isOutput=False)
    nr_d = nc.declare_dram_parameter("nr", [128, 16], f32, isOutput=False)
    ones_d = nc.declare_dram_parameter("ones", [1, 128], mm_dt, isOutput=False)
    r2_d = nc.declare_dram_parameter("r2", [NSEG, 2 * GPC], f32, isOutput=True)

    with tile.TileContext(nc) as tc, ExitStack() as ctx:
        const = ctx.enter_context(tc.tile_pool(name="const", bufs=1))
        psum = ctx.enter_context(tc.tile_pool(name="psum", bufs=4, space="PSUM"))
        dpool = ctx.enter_context(tc.tile_pool(name="d", bufs=4))
        opool = ctx.enter_context(tc.tile_pool(name="o", bufs=2))

        rhs0 = const.tile([128, G], mm_dt, tag="rhs0")
        rhs1 = const.tile([128, G], mm_dt, tag="rhs1")
        nx = const.tile([1, G], mm_dt, tag="nx")
        lhs = {}
        for mod, src in ((0, lhsR_d), (1, lhsI_d)):
            for kb in range(2):
                t = const.tile([128, NSEG], mm_dt, tag=f"lhs{mod}{kb}")
                nc.sync.dma_start(out=t[:], in_=src[kb * 128 : (kb + 1) * 128, :])
                lhs[mod, kb] = t
        nr_t = const.tile([128, 16], f32, tag="nr")
        ones_t = const.tile([1, 128], mm_dt, tag="ones")

        nc.sync.dma_start(out=rhs0[:], in_=rhs_d[0:128, :])
        nc.sync.dma_start(out=rhs1[:], in_=rhs_d[128:256, :])
        nc.sync.dma_start(out=nx[:1, :], in_=rhs_d[256:257, :])
        nc.sync.dma_start(out=nr_t[:], in_=nr_d[:, :])
        # memset can't target float32r tiles (invalid ISA) — DMA ones in.
        nc.sync.dma_start(out=ones_t[:1, :], in_=ones_d[:, :])

        for m in range(8):
            out_t = opool.tile([128, 2 * GPC], f32)
            for mod in range(2):
                bias = nr_t[:, mod * 8 + m : mod * 8 + m + 1]
                for t in range(ntile):
                    cs = bass.ts(t, 512)
                    ps = psum.tile([128, 512], f32)
                    nc.tensor.matmul(
                        ps[:], ones_t[:1, :], nx[:1, cs], start=True, stop=False
                    )
                    nc.tensor.matmul(
                        ps[:], lhs[mod, 0][:, bass.ts(m, 128)], rhs0[:, cs],
                        start=False, stop=False,
                    )
                    nc.tensor.matmul(
                        ps[:], lhs[mod, 1][:, bass.ts(m, 128)], rhs1[:, cs],
                        start=False, stop=True,
                    )
                    d_t = dpool.tile([128, 512], f32)
                    # d = max(psum + nr[g], 1e-12); singleton-label centers
                    # equal their point, so psum+nr can round below zero.
                    nc.vector.tensor_scalar(
                        d_t[:], ps[:], bias, 1e-12,
                        op0=mybir.AluOpType.add, op1=mybir.AluOpType.max,
                    )
                    nc.scalar.activation(
                        d_t[:], d_t[:], mybir.ActivationFunctionType.Sqrt,
                    )
                    oc = mod * GPC + t * gtile
                    nc.vector.tensor_reduce(
                        out_t[:, oc : oc + gtile],
                        d_t[:].rearrange("p (g w) -> p g w", w=Pw),
                        axis=mybir.AxisListType.X,
                        op=mybir.AluOpType.add,
                    )
            nc.sync.dma_start(out=r2_d[bass.ts(m, 128), :], in_=out_t[:])
    # Bacc defers register allocation to finalize(); serialize-after-
    # finalize or walrus sees reg_id=-1.
    nc.finalize()
    return nc


def _seg_mean(x_half: np.ndarray, t_half: np.ndarray):
    """f64 segment mean matching jax.ops.segment_sum + max(count,1) divide."""
    cnt = np.bincount(t_half, minlength=NSEG)
    sums = np.zeros((NSEG, D), np.float64)
    order = np.argsort(t_half, kind="stable")
    xs = x_half[order].astype(np.float64)
    ts_sorted = t_half[order]
    present = np.nonzero(cnt)[0]
    if len(present):
        starts = np.searchsorted(ts_sorted, present)
        sums[present] = np.add.reduceat(xs, starts, axis=0)
    return (sums / np.maximum(cnt, 1)[:, None]).astype(np.float32), cnt


def prepare(inputs: np.ndarray, targets: np.ndarray):
    """Host data marshaling: centers, sorted/padded rhs, per-core in_maps."""
    x = np.asarray(inputs, np.float32)
    t = np.asarray(targets)
    centerR, _ = _seg_mean(x[:HALF], t[:HALF])
    centerI, _ = _seg_mean(x[HALF:], t[HALF:])
    nrR = np.sum(centerR.astype(np.float64) ** 2, axis=1).astype(np.float32)
    nrI = np.sum(centerI.astype(np.float64) ** 2, axis=1).astype(np.float32)
    n_x = np.sum(x.astype(np.float64) ** 2, axis=1).astype(np.float32)

    cnt_all = np.bincount(t, minlength=NSEG)
    maxc = int(cnt_all.max())
    Pw = 4
    while Pw < maxc:
        Pw *= 2
    assert Pw <= 512, f"label group of {maxc} too large"
    Gt = NSEG * Pw
    G = Gt // NCORES

    starts_pos = np.concatenate([[0], np.cumsum(cnt_all)])[:-1]
    order_all = np.argsort(t, kind="stable")
    ts_all = t[order_all]
    dest = ts_all * Pw + (np.arange(N) - starts_pos[ts_all])
    RHS = np.zeros((257, Gt), np.float32)
    RHS[0:256, dest] = x[order_all].T
    RHS[256, dest] = n_x[order_all]
    npad = (Pw - cnt_all).astype(np.float64)

    nr_dev = np.zeros((128, 16), np.float32)
    for m in range(8):
        nr_dev[:, m] = nrR[m * 128 : (m + 1) * 128]
        nr_dev[:, 8 + m] = nrI[m * 128 : (m + 1) * 128]

    lhsR_dev = np.ascontiguousarray((-2.0 * centerR.T).astype(np.float32))
    lhsI_dev = np.ascontiguousarray((-2.0 * centerI.T).astype(np.float32))
    in_maps = [
        {
            "rhs": np.ascontiguousarray(RHS[:, c * G : (c + 1) * G]),
            "lhsR": lhsR_dev,
            "lhsI": lhsI_dev,
            "nr": nr_dev,
            "ones": np.ones((1, 128), np.float32),
        }
        for c in range(NCORES)
    ]
    host = dict(
        centerR=centerR, centerI=centerI, nrR=nrR, nrI=nrI,
        cnt_all=cnt_all, npad=npad, G=G, Pw=Pw, targets=t,
    )
    return in_maps, host


def finish(core_outs, host) -> np.float32:
    """Assemble R2 shards, pad-correct, and reduce to the scalar loss (f64)."""
    t = host["targets"]
    R2R = np.empty((NSEG, NSEG), np.float64)
    R2I = np.empty((NSEG, NSEG), np.float64)
    for c in range(NCORES):
        R2R[:, c * GPC : (c + 1) * GPC] = core_outs[c][:, :GPC]
        R2I[:, c * GPC : (c + 1) * GPC] = core_outs[c][:, GPC:]
    sqrtR = np.sqrt(host["nrR"].astype(np.float64))
    sqrtI = np.sqrt(host["nrI"].astype(np.float64))
    R2R -= sqrtR[:, None] * host["npad"][None, :]
    R2I -= sqrtI[:, None] * host["npad"][None, :]
    rowsumR = R2R.sum(axis=1)
    rowsumI = R2I.sum(axis=1)

    a = 1.0 / (N - host["cnt_all"][t]).astype(np.float64)
    # cR2[i] = centerR[tR[i mod half]] but cI2[i] = centerI[tI[i mod half]]
    gqR = t[np.arange(N) % HALF]
    gqI = t[HALF + (np.arange(N) % HALF)]
    sumR = float(np.sum(a * (rowsumR[gqR] - R2R[gqR, t])))
    sumI = float(np.sum(a * (rowsumI[gqI] - R2I[gqI, t])))

    diff = host["centerR"][t[:HALF]].astype(np.float64) - host["centerI"][
        t[HALF:]
    ].astype(np.float64)
    s_pc = float(np.sum(np.sqrt(np.sum(diff * diff, axis=1))))
    return np.float32(s_pc / (sumR + sumI - s_pc))


def kernel(inputs: np.ndarray, targets: np.ndarray) -> np.ndarray:
    global last_result
    in_maps, host = prepare(inputs, targets)
    key = (host["G"], host["Pw"], MM_DT)
    if key not in _nc_cache:
        _nc_cache[key] = build_nc(host["G"], host["Pw"], MM_DT)
    nc = _nc_cache[key]
    res = run_bass_kernel_spmd(nc, in_maps, list(range(NCORES)))
    last_result = res
    outs = [res.results[c]["r2"] for c in range(NCORES)]
    return finish(outs, host)


# revision 13
# speedup vs baseline: 1.7279x; 1.2180x over previous
"""CenterPNLoss on 8 TRN2 NeuronCores.

Math: the reference builds two 8192x8192 distance matrices between
per-row class centers and all points, then does masked row reductions.
Both matrices have only <=1024 unique rows (one per identity g), and the
masked sums only ever need, for each (center g, label h), the sum of
distances from center g to all points with label h:

    R2[g, h] = sum_{j: targets[j]==h} sqrt(||c_g||^2 + ||x_j||^2 - 2 c_g.x_j)

From R2 (shape [1024, 1024], per modality) every reference quantity is a
cheap gather/sum over 8192 rows, done on the host in f64.

Device work per core (label-sharded: core c owns labels [128c, 128c+128)):
  columns = points sorted by label, zero-padded to Pw per label group.
  psum[g, j] = n_x[j] (K=1 matmul vs ones) - 2 c_g.x_j (two K=128 matmuls)
  d = ACT Sqrt(psum + bias=||c_g||^2)   (per-partition bias)
  R2 chunk = DVE grouped reduce of d over each Pw-wide label group.
Pad columns (x=0, n_x=0) contribute sqrt(||c_g||^2) exactly; the host
subtracts npad[h]*sqrt(nr[g]) afterwards.

No clip-before-sqrt on device: d >= ~100 for randn data with mean-of-4
centers, and pad columns give exactly nr[g] >= 0, so NaN is impossible.
"""

import os
import sys
from contextlib import ExitStack

import numpy as np

sys.path.insert(0, "/opt/trn_rl_repo")

import concourse.bass as bass
import concourse.tile as tile
from concourse import bacc, mybir
from concourse.bass_utils import run_bass_kernel_spmd

N = 8192
D = 256
HALF = N // 2
NSEG = 1024
NCORES = 8
GPC = NSEG // NCORES  # label groups per core: 128

# Matmul operand dtype. Measured on HW: float32 = 4 cyc/row, float32r
# ~1.75 cyc/row; bfloat16 = 1 cyc/row and half-size weight loads. bf16
# operand rounding contributes ~2e-5 relative error on the loss.
MM_DT = mybir.dt.bfloat16

_nc_cache: dict = {}
last_result = None  # BassKernelResults of the most recent run (for test.py)


def build_nc(G: int, Pw: int, mm_dt=MM_DT):
    """One-core SPMD program: [257, G] rhs shard -> [1024, 256] R2 shard."""
    assert G % 512 == 0 and 512 % Pw == 0
    ntile = G // 512       # 512-column tiles
    gtile = 512 // Pw      # label groups per column tile
    f32 = mybir.dt.float32

    # Bacc (not plain Bass): its finalize() runs move_matmul_waits_to_
    # ldweights + generate_event_semaphores, without which walrus rejects
    # Tile-scheduled matmuls ("Too many sync wait commands").
    nc = bacc.Bacc()
    rhs_d = nc.declare_dram_parameter("rhs", [257, G], mm_dt, isOutput=False)
    lhsR_d = nc.declare_dram_parameter("lhsR", [D, NSEG], mm_dt, isOutput=False)
    lhsI_d = nc.declare_dram_parameter("lhsI", [D, NSEG], mm_dt, isOutput=False)
    nr_d = nc.declare_dram_parameter("nr", [128, 16], f32, isOutput=False)
    ones_d = nc.declare_dram_parameter("ones", [1, 128], mm_dt, isOutput=False)
    r2_d = nc.declare_dram_parameter("r2", [NSEG, 2 * GPC], f32, isOutput=True)

    with tile.TileContext(nc) as tc, ExitStack() as ctx:
        const = ctx.enter_context(tc.tile_pool(name="const", bufs=1))
        psum = ctx.enter_context(tc.tile_pool(name="psum", bufs=4, space="PSUM"))
        dpool = ctx.enter_context(tc.tile_pool(name="d", bufs=4))
        opool = ctx.enter_context(tc.tile_pool(name="o", bufs=2))

        rhs0 = const.tile([128, G], mm_dt, tag="rhs0")
        rhs1 = const.tile([128, G], mm_dt, tag="rhs1")
        nx = const.tile([1, G], mm_dt, tag="nx")
        lhs = {}
        for mod, src in ((0, lhsR_d), (1, lhsI_d)):
            for kb in range(2):
                t = const.tile([128, NSEG], mm_dt, tag=f"lhs{mod}{kb}")
                nc.sync.dma_start(out=t[:], in_=src[kb * 128 : (kb + 1) * 128, :])
                lhs[mod, kb] = t
        nr_t = const.tile([128, 16], f32, tag="nr")
        ones_t = const.tile([1, 128], mm_dt, tag="ones")

        nc.sync.dma_start(out=rhs0[:], in_=rhs_d[0:128, :])
        nc.sync.dma_start(out=rhs1[:], in_=rhs_d[128:256, :])
        nc.sync.dma_start(out=nx[:1, :], in_=rhs_d[256:257, :])
        nc.sync.dma_start(out=nr_t[:], in_=nr_d[:, :])
        # memset can't target float32r tiles (invalid ISA) — DMA ones in.
        nc.sync.dma_start(out=ones_t[:1, :], in_=ones_d[:, :])

        for m in range(8):
            out_t = opool.tile([128, 2 * GPC], f32)
            for mod in range(2):
                bias = nr_t[:, mod * 8 + m : mod * 8 + m + 1]
                for t in range(ntile):
                    cs = bass.ts(t, 512)
                    ps = psum.tile([128, 512], f32)
                    nc.tensor.matmul(
                        ps[:], ones_t[:1, :], nx[:1, cs], start=True, stop=False
                    )
                    nc.tensor.matmul(
                        ps[:], lhs[mod, 0][:, bass.ts(m, 128)], rhs0[:, cs],
                        start=False, stop=False,
                    )
                    nc.tensor.matmul(
                        ps[:], lhs[mod, 1][:, bass.ts(m, 128)], rhs1[:, cs],
                        start=False, stop=True,
                    )
                    d_t = dpool.tile([128, 512], f32)
                    # d = max(psum + nr[g], 1e-12); singleton-label centers
                    # equal their point, so psum+nr can round below zero.
                    nc.vector.tensor_scalar(
                        d_t[:], ps[:], bias, 1e-12,
                        op0=mybir.AluOpType.add, op1=mybir.AluOpType.max,
                    )
                    nc.scalar.activation(
                        d_t[:], d_t[:], mybir.ActivationFunctionType.Sqrt,
                    )
                    oc = mod * GPC + t * gtile
                    nc.vector.tensor_reduce(
                        out_t[:, oc : oc + gtile],
                        d_t[:].rearrange("p (g w) -> p g w", w=Pw),
                        axis=mybir.AxisListType.X,
                        op=mybir.AluOpType.add,
                    )
            nc.sync.dma_start(out=r2_d[bass.ts(m, 128), :], in_=out_t[:])
    # Bacc defers register allocation to finalize(); serialize-after-
    # finalize or walrus sees reg_id=-1.
    nc.finalize()
    return nc


def _seg_mean(x_half: np.ndarray, t_half: np.ndarray):
    """f64 segment mean matching jax.ops.segment_sum + max(count,1) divide."""
    cnt = np.bincount(t_half, minlength=NSEG)
    sums = np.zeros((NSEG, D), np.float64)
    order = np.argsort(t_half, kind="stable")
    xs = x_half[order].astype(np.float64)
    ts_sorted = t_half[order]
    present = np.nonzero(cnt)[0]
    if len(present):
        starts = np.searchsorted(ts_sorted, present)
        sums[present] = np.add.reduceat(xs, starts, axis=0)
    return (sums / np.maximum(cnt, 1)[:, None]).astype(np.float32), cnt


def prepare(inputs: np.ndarray, targets: np.ndarray):
    """Host data marshaling: centers, sorted/padded rhs, per-core in_maps."""
    x = np.asarray(inputs, np.float32)
    t = np.asarray(targets)
    centerR, _ = _seg_mean(x[:HALF], t[:HALF])
    centerI, _ = _seg_mean(x[HALF:], t[HALF:])
    nrR = np.sum(centerR.astype(np.float64) ** 2, axis=1).astype(np.float32)
    nrI = np.sum(centerI.astype(np.float64) ** 2, axis=1).astype(np.float32)
    n_x = np.sum(x.astype(np.float64) ** 2, axis=1).astype(np.float32)

    cnt_all = np.bincount(t, minlength=NSEG)
    maxc = int(cnt_all.max())
    Pw = 4
    while Pw < maxc:
        Pw *= 2
    assert Pw <= 512, f"label group of {maxc} too large"
    Gt = NSEG * Pw
    G = Gt // NCORES

    starts_pos = np.concatenate([[0], np.cumsum(cnt_all)])[:-1]
    order_all = np.argsort(t, kind="stable")
    ts_all = t[order_all]
    dest = ts_all * Pw + (np.arange(N) - starts_pos[ts_all])
    RHS = np.zeros((257, Gt), np.float32)
    RHS[0:256, dest] = x[order_all].T
    RHS[256, dest] = n_x[order_all]
    npad = (Pw - cnt_all).astype(np.float64)

    nr_dev = np.zeros((128, 16), np.float32)
    for m in range(8):
        nr_dev[:, m] = nrR[m * 128 : (m + 1) * 128]
        nr_dev[:, 8 + m] = nrI[m * 128 : (m + 1) * 128]

    mm_np = mybir.dt.np(MM_DT)
    lhsR_dev = np.ascontiguousarray((-2.0 * centerR.T).astype(mm_np))
    lhsI_dev = np.ascontiguousarray((-2.0 * centerI.T).astype(mm_np))
    in_maps = [
        {
            "rhs": np.ascontiguousarray(RHS[:, c * G : (c + 1) * G]).astype(mm_np),
            "lhsR": lhsR_dev,
            "lhsI": lhsI_dev,
            "nr": nr_dev,
            "ones": np.ones((1, 128), mm_np),
        }
        for c in range(NCORES)
    ]
    host = dict(
        centerR=centerR, centerI=centerI, nrR=nrR, nrI=nrI,
        cnt_all=cnt_all, npad=npad, G=G, Pw=Pw, targets=t,
    )
    return in_maps, host


def finish(core_outs, host) -> np.float32:
    """Assemble R2 shards, pad-correct, and reduce to the scalar loss (f64)."""
    t = host["targets"]
    R2R = np.empty((NSEG, NSEG), np.float64)
    R2I = np.empty((NSEG, NSEG), np.float64)
    for c in range(NCORES):
        R2R[:, c * GPC : (c + 1) * GPC] = core_outs[c][:, :GPC]
        R2I[:, c * GPC : (c + 1) * GPC] = core_outs[c][:, GPC:]
    sqrtR = np.sqrt(host["nrR"].astype(np.float64))
    sqrtI = np.sqrt(host["nrI"].astype(np.float64))
    R2R -= sqrtR[:, None] * host["npad"][None, :]
    R2I -= sqrtI[:, None] * host["npad"][None, :]
    rowsumR = R2R.sum(axis=1)
    rowsumI = R2I.sum(axis=1)

    a = 1.0 / (N - host["cnt_all"][t]).astype(np.float64)
    # cR2[i] = centerR[tR[i mod half]] but cI2[i] = centerI[tI[i mod half]]
    gqR = t[np.arange(N) % HALF]
    gqI = t[HALF + (np.arange(N) % HALF)]
    sumR = float(np.sum(a * (rowsumR[gqR] - R2R[gqR, t])))
    sumI = float(np.sum(a * (rowsumI[gqI] - R2I[gqI, t])))

    diff = host["centerR"][t[:HALF]].astype(np.float64) - host["centerI"][
        t[HALF:]
    ].astype(np.float64)
    s_pc = float(np.sum(np.sqrt(np.sum(diff * diff, axis=1))))
    return np.float32(s_pc / (sumR + sumI - s_pc))


def kernel(inputs: np.ndarray, targets: np.ndarray) -> np.ndarray:
    global last_result
    in_maps, host = prepare(inputs, targets)
    key = (host["G"], host["Pw"], MM_DT)
    if key not in _nc_cache:
        _nc_cache[key] = build_nc(host["G"], host["Pw"], MM_DT)
    nc = _nc_cache[key]
    res = run_bass_kernel_spmd(nc, in_maps, list(range(NCORES)))
    last_result = res
    outs = [res.results[c]["r2"] for c in range(NCORES)]
    return finish(outs, host)


# revision 18
# speedup vs baseline: 1.8156x; 1.0508x over previous
"""CenterPNLoss on 8 TRN2 NeuronCores.

Math: the reference builds two 8192x8192 distance matrices between
per-row class centers and all points, then does masked row reductions.
Both matrices have only <=1024 unique rows (one per identity g), and the
masked sums only ever need, for each (center g, label h), the sum of
distances from center g to all points with label h:

    R2[g, h] = sum_{j: targets[j]==h} sqrt(||c_g||^2 + ||x_j||^2 - 2 c_g.x_j)

From R2 (shape [1024, 1024], per modality) every reference quantity is a
cheap gather/sum over 8192 rows, done on the host in f64.

Device work per core (label-sharded: core c owns labels [128c, 128c+128)):
  columns = points sorted by label, zero-padded to Pw per label group.
  psum[g, j] = n_x[j] (K=1 matmul vs ones) - 2 c_g.x_j (two K=128 matmuls)
  d = ACT Sqrt(psum + bias=||c_g||^2)   (per-partition bias)
  R2 chunk = DVE grouped reduce of d over each Pw-wide label group.
Pad columns (x=0, n_x=0) contribute sqrt(||c_g||^2) exactly; the host
subtracts npad[h]*sqrt(nr[g]) afterwards.

No clip-before-sqrt on device: d >= ~100 for randn data with mean-of-4
centers, and pad columns give exactly nr[g] >= 0, so NaN is impossible.
"""

import os
import sys
from contextlib import ExitStack

import numpy as np

sys.path.insert(0, "/opt/trn_rl_repo")

import concourse.bass as bass
import concourse.tile as tile
from concourse import bacc, mybir
from concourse.bass_utils import run_bass_kernel_spmd

N = 8192
D = 256
HALF = N // 2
NSEG = 1024
NCORES = 8
GPC = NSEG // NCORES  # label groups per core: 128

# Matmul operand dtype. Measured on HW: float32 = 4 cyc/row, float32r
# ~1.75 cyc/row; bfloat16 = 1 cyc/row and half-size weight loads. bf16
# operand rounding contributes ~2e-5 relative error on the loss.
MM_DT = mybir.dt.bfloat16

_nc_cache: dict = {}
last_result = None  # BassKernelResults of the most recent run (for test.py)


def build_nc(G: int, Pw: int, mm_dt=MM_DT, fast: bool = True):
    """One-core SPMD program: [257, G] rhs shard -> [1024, 256] R2 shard.

    fast=True: no clamp before sqrt — valid when no label is a singleton
    in either half (then no center coincides with a data point and all
    true distances are far from zero; pad columns give exactly nr >= 0).
    fast=False: DVE add+max clamp at 1e-12, matching the reference clip.
    """
    assert G % 512 == 0 and 512 % Pw == 0
    ntile = G // 512       # 512-column tiles
    gtile = 512 // Pw      # label groups per column tile
    f32 = mybir.dt.float32
    bf16 = mybir.dt.bfloat16
    TCH = min(ntile, 4)    # column tiles per psum batch (<=4 of 8 banks)

    # Bacc (not plain Bass): its finalize() runs move_matmul_waits_to_
    # ldweights + generate_event_semaphores, without which walrus rejects
    # Tile-scheduled matmuls ("Too many sync wait commands").
    nc = bacc.Bacc()
    rhs_d = nc.declare_dram_parameter("rhs", [257, G], mm_dt, isOutput=False)
    lhsR_d = nc.declare_dram_parameter("lhsR", [D, NSEG], mm_dt, isOutput=False)
    lhsI_d = nc.declare_dram_parameter("lhsI", [D, NSEG], mm_dt, isOutput=False)
    nr_d = nc.declare_dram_parameter("nr", [128, 16], f32, isOutput=False)
    ones_d = nc.declare_dram_parameter("ones", [1, 128], mm_dt, isOutput=False)
    r2_d = nc.declare_dram_parameter("r2", [NSEG, 2 * GPC], f32, isOutput=True)

    with tile.TileContext(nc) as tc, ExitStack() as ctx:
        const = ctx.enter_context(tc.tile_pool(name="const", bufs=1))
        psum = ctx.enter_context(tc.tile_pool(name="psum", bufs=2, space="PSUM"))
        dpool = ctx.enter_context(tc.tile_pool(name="d", bufs=6))
        opool = ctx.enter_context(tc.tile_pool(name="o", bufs=2))

        rhs0 = const.tile([128, G], mm_dt, tag="rhs0")
        rhs1 = const.tile([128, G], mm_dt, tag="rhs1")
        nx = const.tile([1, G], mm_dt, tag="nx")
        lhs = {}
        for mod, src in ((0, lhsR_d), (1, lhsI_d)):
            for kb in range(2):
                t = const.tile([128, NSEG], mm_dt, tag=f"lhs{mod}{kb}")
                nc.sync.dma_start(out=t[:], in_=src[kb * 128 : (kb + 1) * 128, :])
                lhs[mod, kb] = t
        nr_t = const.tile([128, 16], f32, tag="nr")
        ones_t = const.tile([1, 128], mm_dt, tag="ones")

        nc.sync.dma_start(out=rhs0[:], in_=rhs_d[0:128, :])
        nc.sync.dma_start(out=rhs1[:], in_=rhs_d[128:256, :])
        nc.sync.dma_start(out=nx[:1, :], in_=rhs_d[256:257, :])
        nc.sync.dma_start(out=nr_t[:], in_=nr_d[:, :])
        # memset can't target float32r tiles (invalid ISA) — DMA ones in.
        nc.sync.dma_start(out=ones_t[:1, :], in_=ones_d[:, :])

        for m in range(8):
            out_t = opool.tile([128, 2 * GPC], f32)
            for mod in range(2):
                bias = nr_t[:, mod * 8 + m : mod * 8 + m + 1]
                for tb in range(0, ntile, TCH):
                    tcur = range(tb, min(tb + TCH, ntile))
                    ps = {t: psum.tile([128, 512], f32, tag=f"ps{t - tb}",
                                       name=f"ps_{m}_{mod}_{t}")
                          for t in tcur}
                    # group matmuls by stationary operand so consecutive
                    # instructions reuse the loaded weights
                    for t in tcur:
                        nc.tensor.matmul(
                            ps[t][:], ones_t[:1, :], nx[:1, bass.ts(t, 512)],
                            start=True, stop=False,
                        )
                    for kb, rhs_t in ((0, rhs0), (1, rhs1)):
                        w = lhs[mod, kb][:, bass.ts(m, 128)]
                        for t in tcur:
                            nc.tensor.matmul(
                                ps[t][:], w, rhs_t[:, bass.ts(t, 512)],
                                start=False, stop=(kb == 1),
                            )
                    for t in tcur:
                        oc = mod * GPC + t * gtile
                        if fast:
                            d_t = dpool.tile([128, 512], bf16, tag="d")
                            nc.scalar.activation(
                                d_t[:], ps[t][:],
                                mybir.ActivationFunctionType.Sqrt,
                                bias=bias, scale=1.0,
                            )
                        else:
                            d_t = dpool.tile([128, 512], f32, tag="d")
                            nc.vector.tensor_scalar(
                                d_t[:], ps[t][:], bias, 1e-12,
                                op0=mybir.AluOpType.add,
                                op1=mybir.AluOpType.max,
                            )
                            nc.scalar.activation(
                                d_t[:], d_t[:],
                                mybir.ActivationFunctionType.Sqrt,
                            )
                        nc.vector.tensor_reduce(
                            out_t[:, oc : oc + gtile],
                            d_t[:].rearrange("p (g w) -> p g w", w=Pw),
                            axis=mybir.AxisListType.X,
                            op=mybir.AluOpType.add,
                        )
            nc.sync.dma_start(out=r2_d[bass.ts(m, 128), :], in_=out_t[:])
    # Bacc defers register allocation to finalize(); serialize-after-
    # finalize or walrus sees reg_id=-1.
    nc.finalize()
    return nc


def _seg_mean(x_half: np.ndarray, t_half: np.ndarray):
    """f64 segment mean matching jax.ops.segment_sum + max(count,1) divide."""
    cnt = np.bincount(t_half, minlength=NSEG)
    sums = np.zeros((NSEG, D), np.float64)
    order = np.argsort(t_half, kind="stable")
    xs = x_half[order].astype(np.float64)
    ts_sorted = t_half[order]
    present = np.nonzero(cnt)[0]
    if len(present):
        starts = np.searchsorted(ts_sorted, present)
        sums[present] = np.add.reduceat(xs, starts, axis=0)
    return (sums / np.maximum(cnt, 1)[:, None]).astype(np.float32), cnt


def prepare(inputs: np.ndarray, targets: np.ndarray):
    """Host data marshaling: centers, sorted/padded rhs, per-core in_maps."""
    x = np.asarray(inputs, np.float32)
    t = np.asarray(targets)
    centerR, _ = _seg_mean(x[:HALF], t[:HALF])
    centerI, _ = _seg_mean(x[HALF:], t[HALF:])
    nrR = np.sum(centerR.astype(np.float64) ** 2, axis=1).astype(np.float32)
    nrI = np.sum(centerI.astype(np.float64) ** 2, axis=1).astype(np.float32)
    n_x = np.sum(x.astype(np.float64) ** 2, axis=1).astype(np.float32)

    cnt_all = np.bincount(t, minlength=NSEG)
    maxc = int(cnt_all.max())
    Pw = 4
    while Pw < maxc:
        Pw *= 2
    assert Pw <= 512, f"label group of {maxc} too large"
    Gt = NSEG * Pw
    G = Gt // NCORES

    starts_pos = np.concatenate([[0], np.cumsum(cnt_all)])[:-1]
    order_all = np.argsort(t, kind="stable")
    ts_all = t[order_all]
    dest = ts_all * Pw + (np.arange(N) - starts_pos[ts_all])
    RHS = np.zeros((257, Gt), np.float32)
    RHS[0:256, dest] = x[order_all].T
    RHS[256, dest] = n_x[order_all]
    npad = (Pw - cnt_all).astype(np.float64)

    nr_dev = np.zeros((128, 16), np.float32)
    for m in range(8):
        nr_dev[:, m] = nrR[m * 128 : (m + 1) * 128]
        nr_dev[:, 8 + m] = nrI[m * 128 : (m + 1) * 128]

    mm_np = mybir.dt.np(MM_DT)
    lhsR_dev = np.ascontiguousarray((-2.0 * centerR.T).astype(mm_np))
    lhsI_dev = np.ascontiguousarray((-2.0 * centerI.T).astype(mm_np))
    in_maps = [
        {
            "rhs": np.ascontiguousarray(RHS[:, c * G : (c + 1) * G]).astype(mm_np),
            "lhsR": lhsR_dev,
            "lhsI": lhsI_dev,
            "nr": nr_dev,
            "ones": np.ones((1, 128), mm_np),
        }
        for c in range(NCORES)
    ]
    cntR = np.bincount(t[:HALF], minlength=NSEG)
    cntI = np.bincount(t[HALF:], minlength=NSEG)
    fast = not ((cntR == 1).any() or (cntI == 1).any())
    host = dict(
        centerR=centerR, centerI=centerI, nrR=nrR, nrI=nrI,
        cnt_all=cnt_all, npad=npad, G=G, Pw=Pw, targets=t, fast=fast,
    )
    return in_maps, host


def finish(core_outs, host) -> np.float32:
    """Assemble R2 shards, pad-correct, and reduce to the scalar loss (f64)."""
    t = host["targets"]
    R2R = np.empty((NSEG, NSEG), np.float64)
    R2I = np.empty((NSEG, NSEG), np.float64)
    for c in range(NCORES):
        R2R[:, c * GPC : (c + 1) * GPC] = core_outs[c][:, :GPC]
        R2I[:, c * GPC : (c + 1) * GPC] = core_outs[c][:, GPC:]
    sqrtR = np.sqrt(host["nrR"].astype(np.float64))
    sqrtI = np.sqrt(host["nrI"].astype(np.float64))
    R2R -= sqrtR[:, None] * host["npad"][None, :]
    R2I -= sqrtI[:, None] * host["npad"][None, :]
    rowsumR = R2R.sum(axis=1)
    rowsumI = R2I.sum(axis=1)

    a = 1.0 / (N - host["cnt_all"][t]).astype(np.float64)
    # cR2[i] = centerR[tR[i mod half]] but cI2[i] = centerI[tI[i mod half]]
    gqR = t[np.arange(N) % HALF]
    gqI = t[HALF + (np.arange(N) % HALF)]
    sumR = float(np.sum(a * (rowsumR[gqR] - R2R[gqR, t])))
    sumI = float(np.sum(a * (rowsumI[gqI] - R2I[gqI, t])))

    diff = host["centerR"][t[:HALF]].astype(np.float64) - host["centerI"][
        t[HALF:]
    ].astype(np.float64)
    s_pc = float(np.sum(np.sqrt(np.sum(diff * diff, axis=1))))
    return np.float32(s_pc / (sumR + sumI - s_pc))


def kernel(inputs: np.ndarray, targets: np.ndarray) -> np.ndarray:
    global last_result
    in_maps, host = prepare(inputs, targets)
    key = (host["G"], host["Pw"], MM_DT, host["fast"])
    if key not in _nc_cache:
        _nc_cache[key] = build_nc(host["G"], host["Pw"], MM_DT, host["fast"])
    nc = _nc_cache[key]
    res = run_bass_kernel_spmd(nc, in_maps, list(range(NCORES)))
    last_result = res
    outs = [res.results[c]["r2"] for c in range(NCORES)]
    return finish(outs, host)
